# revision 2
# baseline (speedup 1.0000x reference)
"""3-layer GCN + pooled MLP head on 8 Trainium2 NeuronCores.

Strategy (dst-sharded message passing):
- Relabel nodes by in-degree (desc), deal round-robin to 8 cores; each core
  owns 6250 dst nodes (padded to 6272 = 49 tiles of 128).
- Per layer: each core computes its slice of hhat = dinv * (y @ W) feature-major
  on PE, transposes to node-major, AllGathers hhat [50176, 64] to DRAM.
- Aggregation: per dst tile, dma_gather pulls single 256B rows from DRAM hhat
  using signed int16 indices against a base shifted by +32768 rows (idx =
  row - 32768 covers all 50176 rows); padding cells point at a known zero row
  in the high region, so a plain strided DVE reduce does the segment sum with
  no masking. Self-loop add + dinv_dst scaling fused; bias+ReLU ride the PE
  transpose through the ACT engine.
- The next layer's feature matmul (mm chunks) is interleaved into the
  aggregation tile loop so PE/DVE work hides under gather descriptor gen,
  and the AllGather launches immediately after the last chunk.
- Head: per-core feature-major sum/max pools, tiny AllGather, replicated MLP.
"""
import os
import sys
import types

sys.path.insert(0, "/opt/trn_rl_repo")

import numpy as np

import concourse.bass as bass
import concourse.bacc as bacc
import concourse.tile as tile
import concourse.mybir as mybir
from concourse import bass_utils

N = 50000
E = 800000
D_IN = 128
H = 64
NC = 8
NPC = 6272          # padded nodes per core (49 tiles of 128)
NT = 49             # dst tiles per core
NTOT = NC * NPC     # 50176 rows in the allgathered hhat
MAXCOLS = 64        # max gather columns (rows per slot) per dma_gather call
SHIFT = 32768       # int16 index base shift
HALF = NPC // 2     # 3136: slots split into two AllGather halves
HOUT = NC * HALF    # 25088 rows per half in ag_out
ZERO_ROW = NC * NPC - 1   # 50175: core 7 slot 6271 (half 1 padding), always zero
ZIDX = ZERO_ROW - SHIFT   # 17407, non-negative (avoids trailing-negative trim)

_EXEC_NS = [None]


def _install_trace_hook():
    try:
        from trn_agent_boot.trn_boot import _ntff_profile_via_ctypes
        hook = _ntff_profile_via_ctypes('/opt/axon/libaxon_pjrt.so')
        if hook is None:
            return False
        mod = types.ModuleType('antenv.axon_hooks')
        mod.get_axon_ntff_profile_hook = lambda: hook
        sys.modules['antenv.axon_hooks'] = mod
        return True
    except Exception:
        return False


def _preprocess(edge_index):
    """Graph partitioning: relabel, shard, per-slot neighbor row lists."""
    src = np.asarray(edge_index[0], np.int64)
    dst = np.asarray(edge_index[1], np.int64)
    deg = np.bincount(dst, minlength=N)          # in-degree (no self loop)
    dinv = (1.0 / np.sqrt(deg + 1.0)).astype(np.float32)

    order = np.argsort(-deg, kind="stable")       # relabel: rank r -> orig order[r]
    rank_of = np.empty(N, np.int64)
    rank_of[order] = np.arange(N)
    core_of = rank_of % NC
    slot_of = rank_of // NC                       # 0..6249, degree-desc within core
    row_of = core_of * NPC + slot_of              # DRAM row in hhat_full

    per_core = []
    src_row = row_of[src]
    dst_core = core_of[dst]
    dst_slot = slot_of[dst]
    for c in range(NC):
        em = dst_core == c
        e_slot = dst_slot[em]
        e_srow = src_row[em]
        o = np.argsort(e_slot, kind="stable")
        e_slot = e_slot[o]
        e_srow = e_srow[o]
        counts = np.bincount(e_slot, minlength=NPC)
        starts = np.concatenate([[0], np.cumsum(counts)])
        tiles = []
        for t in range(NT):
            sl0 = t * 128
            nt = int(counts[sl0:sl0 + 128].max())
            rows_by_slot = []
            for p in range(128):
                s = sl0 + p
                rows_by_slot.append(e_srow[starts[s]:starts[s + 1]]
                                    if s < NPC else np.empty(0, np.int64))
            tiles.append((nt, rows_by_slot))
        per_core.append(tiles)
    return dinv, order, per_core


def _build_groups(nt_max):
    """Group tiles so total columns (incl +1 filler per group) <= MAXCOLS+1."""
    groups = []
    g_tiles, g_cols = [], 0
    for t in range(NT):
        nt = int(nt_max[t])
        if g_tiles and g_cols + nt > MAXCOLS:
            groups.append(g_tiles)
            g_tiles, g_cols = [], 0
        g_tiles.append(t)
        g_cols += nt
    if g_tiles:
        groups.append(g_tiles)
    return groups


def _build_idx(per_core_tiles, nt_max, groups_meta):
    """Flat int16 gather index stream for one core (shifted rows + fillers)."""
    parts = []
    for g in groups_meta:
        for t in g:
            ntu = int(nt_max[t])
            if ntu == 0:
                continue
            nt, rows_by_slot = per_core_tiles[t]
            grid = np.full((ntu, 128), ZIDX, np.int32)
            for p in range(128):
                rows = rows_by_slot[p]
                k = len(rows)
                if k:
                    grid[:k, p] = rows - SHIFT
            parts.append(grid.reshape(-1))
        parts.append(np.full(128, ZIDX, np.int32))   # filler column per group
    flat = np.concatenate(parts).astype(np.int16)
    assert flat[-1] >= 0
    wrap = np.zeros((128, len(flat) // 16), np.int16)
    a = flat.reshape(-1, 16)
    for gg in range(8):
        wrap[gg * 16:(gg + 1) * 16, :] = a.T
    return wrap, len(flat) // 128


def _build_program(tiles_meta, groups_meta, idx_cols16):
    """Build the bass program (same for all cores; per-core data via inputs)."""
    nc = bacc.Bacc("TRN2", target_bir_lowering=False, debug=False, num_devices=NC,
                   num_swdge_queues=4, dynamic_dma_scratch_size=32768)
    f32 = mybir.dt.float32
    xT_d = nc.dram_tensor("xT", [D_IN, NPC], f32, kind="ExternalInput")
    W1_d = nc.dram_tensor("W1", [D_IN, H], f32, kind="ExternalInput")
    W2_d = nc.dram_tensor("W2", [H, H], f32, kind="ExternalInput")
    W3_d = nc.dram_tensor("W3", [H, H], f32, kind="ExternalInput")
    bcol_d = nc.dram_tensor("bcol", [H, 3], f32, kind="ExternalInput")
    dinvc_d = nc.dram_tensor("dinvc", [128, NT], f32, kind="ExternalInput")
    idx_d = nc.dram_tensor("idx16", [128, idx_cols16], mybir.dt.int16, kind="ExternalInput")
    ident_d = nc.dram_tensor("ident", [128, 128], f32, kind="ExternalInput")
    fw1_d = nc.dram_tensor("fw1", [2 * H, H], f32, kind="ExternalInput")
    fb1_d = nc.dram_tensor("fb1", [H, 1], f32, kind="ExternalInput")
    fw2_d = nc.dram_tensor("fw2", [H, 1], f32, kind="ExternalInput")
    fb2_d = nc.dram_tensor("fb2", [1, 1], f32, kind="ExternalInput")
    out_d = nc.dram_tensor("out", [1, 1], f32, kind="ExternalOutput")

    with tile.TileContext(nc) as tc:
        with (
            tc.tile_pool(name="const", bufs=1) as cst,
            tc.tile_pool(name="hhat", bufs=1) as hhp,
            tc.tile_pool(name="yt", bufs=1) as ytp,
            tc.tile_pool(name="gb", bufs=4) as gbp,
            tc.tile_pool(name="acc", bufs=3) as accp,
            tc.tile_pool(name="ps", bufs=2, space="PSUM") as psp,
            tc.tile_pool(name="hdps", bufs=1, space="PSUM") as hdp,
            tc.tile_pool(name="zps", bufs=2, space="PSUM") as zpsp,
            tc.tile_pool(name="zsb", bufs=2) as zsbp,
            tc.tile_pool(name="dram", bufs=1, space="DRAM") as dram,
        ):
            # constants
            W1 = cst.tile([D_IN, H], f32)
            nc.sync.dma_start(out=W1[:], in_=W1_d[:])
            W2 = cst.tile([H, H], f32)
            nc.sync.dma_start(out=W2[:], in_=W2_d[:])
            W3 = cst.tile([H, H], f32)
            nc.sync.dma_start(out=W3[:], in_=W3_d[:])
            Wt = [W1, W2, W3]
            bcol = cst.tile([H, 3], f32)
            nc.sync.dma_start(out=bcol[:], in_=bcol_d[:])
            dinvc = cst.tile([128, NT], f32)
            nc.sync.dma_start(out=dinvc[:], in_=dinvc_d[:])
            idx16 = cst.tile([128, idx_cols16], mybir.dt.int16)
            nc.sync.dma_start(out=idx16[:], in_=idx_d[:])
            ident = cst.tile([128, 128], f32)
            nc.sync.dma_start(out=ident[:], in_=ident_d[:])
            fw1 = cst.tile([2 * H, H], f32)
            nc.sync.dma_start(out=fw1[:], in_=fw1_d[:])
            fb1 = cst.tile([H, 1], f32)
            nc.sync.dma_start(out=fb1[:], in_=fb1_d[:])
            fw2 = cst.tile([H, 1], f32)
            nc.sync.dma_start(out=fw2[:], in_=fw2_d[:])
            fb2 = cst.tile([1, 1], f32)
            nc.sync.dma_start(out=fb2[:], in_=fb2_d[:])

            xT = cst.tile([D_IN, NPC], f32)
            hh = [hhp.tile([128, NT * H], f32, name=f"hh{i}") for i in range(2)]
            yT = ytp.tile([H, NPC], f32)

            ag_in = [dram.tile([NPC, H], f32, name=f"agin{l}") for l in range(3)]
            ag_out = [dram.tile([NTOT, H], f32, addr_space="Shared", name=f"agout{l}")
                      for l in range(3)]

            def mm_chunk(l, rhs_sb, ch0, cw, hob):
                """One 512-col chunk of hhat_l = dinv * (W_l^T @ rhs); write to
                hh[hob] node-major and flush the chunk to ag_in[l]."""
                zps = zpsp.tile([H, 512], f32, tag="zps")
                nc.tensor.matmul(out=zps[:, :cw], lhsT=Wt[l][:],
                                 rhs=rhs_sb[:, ch0:ch0 + cw],
                                 start=True, stop=True)
                zsb = zsbp.tile([H, 512], f32, tag="zsb")
                nc.vector.tensor_copy(out=zsb[:, :cw], in_=zps[:, :cw])
                for q in range(0, cw, 128):
                    t = (ch0 + q) // 128
                    tp = psp.tile([128, H], f32, tag="tp")
                    nc.tensor.transpose(out=tp[:], in_=zsb[:, q:q + 128],
                                        identity=ident[:H, :H])
                    nc.vector.tensor_scalar(
                        out=hh[hob][:, t * H:(t + 1) * H], in0=tp[:],
                        scalar1=dinvc[:, t:t + 1], scalar2=None,
                        op0=mybir.AluOpType.mult)
                t0 = ch0 // 128
                ntc = cw // 128
                nc.sync.dma_start(
                    out=ag_in[l][:].rearrange("(t p) d -> p t d", p=128)
                        [:, t0:t0 + ntc, :],
                    in_=hh[hob][:, t0 * H:(t0 + ntc) * H]
                        .rearrange("p (t d) -> p t d", d=H))

            def launch_ag(l):
                nc.gpsimd.collective_compute(
                    "AllGather", mybir.AluOpType.bypass,
                    replica_groups=[list(range(NC))],
                    ins=[ag_in[l].opt()], outs=[ag_out[l].opt()])

            # ---- prologue: layer 0 mm over xT chunks (overlap load+matmul) ----
            for ch0 in range(0, NPC, 512):
                cw = min(512, NPC - ch0)
                nc.sync.dma_start(out=xT[:, ch0:ch0 + cw],
                                  in_=xT_d[:, ch0:ch0 + cw])
                mm_chunk(0, xT, ch0, cw, 0)
            launch_ag(0)

            sum_acc = accp.tile([H, 1], f32, tag="pool", name="sum_acc")
            max_acc = accp.tile([H, 1], f32, tag="pool", name="max_acc")

            def pool_chunk(ch0, cw, first):
                rs = accp.tile([H, 1], f32, tag="pool", name=f"rs{ch0}")
                rm = accp.tile([H, 1], f32, tag="pool", name=f"rm{ch0}")
                nc.vector.reduce_sum(out=rs[:], in_=yT[:, ch0:ch0 + cw],
                                     axis=mybir.AxisListType.X)
                nc.vector.reduce_max(out=rm[:], in_=yT[:, ch0:ch0 + cw],
                                     axis=mybir.AxisListType.X)
                if first:
                    nc.vector.tensor_copy(out=sum_acc[:], in_=rs[:])
                    nc.vector.tensor_copy(out=max_acc[:], in_=rm[:])
                else:
                    nc.vector.tensor_add(out=sum_acc[:], in0=sum_acc[:], in1=rs[:])
                    nc.vector.tensor_max(out=max_acc[:], in0=max_acc[:], in1=rm[:])

            def aggregate(l):
                """dst-tile aggregation from ag_out[l] into yT; next layer's mm
                chunks (or final pooling chunks) interleave after each 512 cols."""
                hob = l % 2
                src_view = ag_out[l][SHIFT:SHIFT + 128, :]
                col_off = 0
                gi = 0
                next_ch0 = 0
                for g_tiles in groups_meta:
                    cols = sum(tiles_meta[t] for t in g_tiles) + 1  # +filler
                    nidx = cols * 128
                    gb = gbp.tile([128, cols * H], f32, tag="gb",
                                  name=f"gb{l}_{g_tiles[0]}")
                    nc.gpsimd.dma_gather(
                        out_ap=gb[:].rearrange("p (n d) -> p n d", d=H),
                        in_ap=src_view,
                        idxs_ap=idx16[:, col_off * 8:(col_off + cols) * 8],
                        num_idxs=nidx, num_idxs_reg=nidx,
                        elem_size=H, single_packet=False,
                        queue_num=gi % 4)
                    gi += 1
                    seg = 0
                    for t in g_tiles:
                        nt = tiles_meta[t]
                        if nt == 0:
                            continue
                        gseg = gb[:, seg * H:(seg + nt) * H]
                        acc = accp.tile([128, H], f32, tag="acc", name=f"acc{l}_{t}")
                        nc.vector.tensor_reduce(
                            out=acc[:],
                            in_=gseg.rearrange("p (cs d) -> p d cs", d=H),
                            axis=mybir.AxisListType.X, op=mybir.AluOpType.add)
                        nc.vector.tensor_add(out=acc[:], in0=acc[:],
                                             in1=hh[hob][:, t * H:(t + 1) * H])
                        nc.vector.tensor_scalar(
                            out=acc[:], in0=acc[:], scalar1=dinvc[:, t:t + 1],
                            scalar2=None, op0=mybir.AluOpType.mult)
                        yps = psp.tile([H, 128], f32, tag="yps", name=f"yps{l}_{t}")
                        nc.tensor.transpose(out=yps[:], in_=acc[:], identity=ident[:])
                        nc.scalar.activation(
                            out=yT[:, t * 128:(t + 1) * 128], in_=yps[:],
                            func=mybir.ActivationFunctionType.Relu,
                            bias=bcol[:, l:l + 1])
                        seg += nt
                        # interleave next layer's mm (or pooling) on done tiles
                        if l < 2:
                            while next_ch0 + 512 <= (t + 1) * 128:
                                mm_chunk(l + 1, yT, next_ch0, 512, 1 - hob)
                                next_ch0 += 512
                        else:
                            while (next_ch0 + 512 <= (t + 1) * 128
                                   and next_ch0 + 512 <= 6144):
                                pool_chunk(next_ch0, 512, next_ch0 == 0)
                                next_ch0 += 512
                    col_off += cols
                if l < 2:
                    while next_ch0 < NPC:
                        cw = min(512, NPC - next_ch0)
                        mm_chunk(l + 1, yT, next_ch0, cw, 1 - hob)
                        next_ch0 += cw
                    launch_ag(l + 1)
                else:
                    # zero pad columns (slots 6250..6271), pool the tail chunk
                    nc.vector.memset(yT[:, NPC - 22:], 0.0)
                    pool_chunk(6144, NPC - 6144, False)

            aggregate(0)
            aggregate(1)
            aggregate(2)

            pool2 = accp.tile([H, 2], f32, tag="pool", name="pool2")
            nc.vector.tensor_copy(out=pool2[:, 0:1], in_=sum_acc[:])
            nc.vector.tensor_copy(out=pool2[:, 1:2], in_=max_acc[:])
            agp_in = dram.tile([H, 2], f32, name="agpin")
            agp_out = dram.tile([NC * H, 2], f32, addr_space="Shared", name="agpout")
            nc.sync.dma_start(out=agp_in[:], in_=pool2[:])
            nc.gpsimd.collective_compute(
                "AllGather", mybir.AluOpType.bypass,
                replica_groups=[list(range(NC))],
                ins=[agp_in.opt()], outs=[agp_out.opt()])
            allp = accp.tile([H, 2 * NC], f32, tag="allp", name="allp")
            nc.sync.dma_start(
                out=allp[:].rearrange("p (r d) -> p r d", d=2),
                in_=agp_out[:].rearrange("(r p) d -> p r d", p=H))
            gsum = accp.tile([H, 1], f32, tag="pool", name="gsum")
            gmax = accp.tile([H, 1], f32, tag="pool", name="gmax")
            nc.vector.reduce_sum(
                out=gsum[:], in_=allp[:].rearrange("p (r d) -> p d r", d=2)[:, 0:1, :],
                axis=mybir.AxisListType.X)
            nc.vector.reduce_max(
                out=gmax[:], in_=allp[:].rearrange("p (r d) -> p d r", d=2)[:, 1:2, :],
                axis=mybir.AxisListType.X)
            nc.vector.tensor_scalar(out=gsum[:], in0=gsum[:], scalar1=1.0 / N,
                                    scalar2=None, op0=mybir.AluOpType.mult)
            pooled = accp.tile([2 * H, 1], f32, tag="pooled", name="pooled")
            nc.sync.dma_start(out=pooled[:H, :], in_=gsum[:])
            nc.sync.dma_start(out=pooled[H:, :], in_=gmax[:])
            h1ps = hdp.tile([H, 1], f32, tag="hd", name="h1ps")
            nc.tensor.matmul(out=h1ps[:], lhsT=fw1[:], rhs=pooled[:],
                             start=True, stop=True)
            r1 = accp.tile([H, 1], f32, tag="pool", name="r1")
            nc.scalar.activation(out=r1[:], in_=h1ps[:],
                                 func=mybir.ActivationFunctionType.Relu,
                                 bias=fb1[:, 0:1])
            h2ps = hdp.tile([1, 1], f32, tag="hd2", name="h2ps")
            nc.tensor.matmul(out=h2ps[:], lhsT=fw2[:], rhs=r1[:],
                             start=True, stop=True)
            ores = accp.tile([1, 1], f32, tag="ores", name="ores")
            nc.vector.tensor_add(out=ores[:], in0=h2ps[:],
                                 in1=fb2[:, 0:1])
            nc.sync.dma_start(out=out_d[:], in_=ores[:])

    nc.compile()
    return nc


def kernel(x, edge_index, W1, b1, W2, b2, W3, b3, fw1, fb1, fw2, fb2):
    dinv, order, per_core = _preprocess(edge_index)

    # Unified per-tile column counts across cores (one SPMD program).
    nts = np.zeros((NC, NT), np.int64)
    for c in range(NC):
        for t in range(NT):
            nts[c, t] = per_core[c][t][0]
    nt_max = nts.max(axis=0)
    tiles_meta = [int(nt_max[t]) for t in range(NT)]
    groups_meta = _build_groups(nt_max)
    total_cols = int(nt_max.sum()) + len(groups_meta)   # + fillers
    idx_cols16 = total_cols * 8

    b3col = np.stack([np.asarray(b1, np.float32), np.asarray(b2, np.float32),
                      np.asarray(b3, np.float32)], axis=1)  # [64, 3]
    ident = np.eye(128, dtype=np.float32)

    in_maps = []
    x = np.asarray(x, np.float32)
    for c in range(NC):
        ranks = np.arange(c, N, NC)
        orig = order[ranks]
        xT = np.zeros((D_IN, NPC), np.float32)
        xT[:, :len(orig)] = x[orig].T
        dv = np.zeros(NPC, np.float32)
        dv[:len(orig)] = dinv[orig]
        dinv_col = dv.reshape(NT, 128).T.copy()      # [128, 49]
        wrap, ncols = _build_idx(per_core[c], nt_max, groups_meta)
        assert ncols == total_cols, (ncols, total_cols)
        in_maps.append({
            "xT": xT, "W1": np.asarray(W1, np.float32),
            "W2": np.asarray(W2, np.float32), "W3": np.asarray(W3, np.float32),
            "bcol": b3col, "dinvc": dinv_col, "idx16": wrap,
            "ident": ident, "fw1": np.asarray(fw1, np.float32),
            "fb1": np.asarray(fb1, np.float32).reshape(H, 1),
            "fw2": np.asarray(fw2, np.float32).reshape(H, 1),
            "fb2": np.asarray(fb2, np.float32).reshape(1, 1),
        })

    nc = _build_program(tiles_meta, groups_meta, idx_cols16)

    trace = os.environ.get("BASS_GCN_TRACE", "0") == "1"
    if trace:
        trace = _install_trace_hook()
    res = bass_utils.run_bass_kernel_spmd(
        nc, in_maps, core_ids=list(range(NC)), trace=trace)
    _EXEC_NS[0] = res.exec_time_ns
    out = res.results[0]["out"]
    return np.asarray(out, np.float32).reshape(1, 1)



# revision 5
# speedup vs baseline: 1.0731x; 1.0731x over previous
"""3-layer GCN + pooled MLP head on 8 Trainium2 NeuronCores.

Strategy (dst-sharded message passing):
- Relabel nodes by in-degree (desc), deal round-robin to 8 cores; each core
  owns 6250 dst nodes (padded to 6272 = 49 tiles of 128).
- Per layer: each core computes its slice of hhat = dinv * (y @ W) feature-major
  on PE, transposes to node-major, AllGathers hhat [50176, 64] to DRAM.
- Aggregation: per dst tile, dma_gather pulls single 256B rows from DRAM hhat
  using signed int16 indices against a base shifted by +32768 rows (idx =
  row - 32768 covers all 50176 rows); padding cells point at a known zero row
  in the high region, so a plain strided DVE reduce does the segment sum with
  no masking. Self-loop add + dinv_dst scaling fused; bias+ReLU ride the PE
  transpose through the ACT engine.
- The next layer's feature matmul (mm chunks) is interleaved into the
  aggregation tile loop so PE/DVE work hides under gather descriptor gen,
  and the AllGather launches immediately after the last chunk.
- Head: per-core feature-major sum/max pools, tiny AllGather, replicated MLP.
"""
import os
import sys
import types

sys.path.insert(0, "/opt/trn_rl_repo")

import numpy as np

import concourse.bass as bass
import concourse.bacc as bacc
import concourse.tile as tile
import concourse.mybir as mybir
from concourse import bass_utils

N = 50000
E = 800000
D_IN = 128
H = 64
NC = 8
NPC = 6272          # padded nodes per core (49 tiles of 128)
NT = 49             # dst tiles per core
NTOT = NC * NPC     # 50176 rows in the allgathered hhat
MAXCOLS = 96        # max gather columns (rows per slot) per dma_gather call
SHIFT = 32768       # int16 index base shift
HALF = NPC // 2     # 3136: slots split into two AllGather halves
HOUT = NC * HALF    # 25088 rows per half in ag_out
ZERO_ROW = NC * NPC - 1   # 50175: core 7 slot 6271 (half 1 padding), always zero
ZIDX = ZERO_ROW - SHIFT   # 17407, non-negative (avoids trailing-negative trim)

_EXEC_NS = [None]


def _install_trace_hook():
    try:
        from trn_agent_boot.trn_boot import _ntff_profile_via_ctypes
        hook = _ntff_profile_via_ctypes('/opt/axon/libaxon_pjrt.so')
        if hook is None:
            return False
        mod = types.ModuleType('antenv.axon_hooks')
        mod.get_axon_ntff_profile_hook = lambda: hook
        sys.modules['antenv.axon_hooks'] = mod
        return True
    except Exception:
        return False


def _preprocess(edge_index):
    """Graph partitioning: relabel, shard, per-slot neighbor row lists."""
    src = np.asarray(edge_index[0], np.int64)
    dst = np.asarray(edge_index[1], np.int64)
    deg = np.bincount(dst, minlength=N)          # in-degree (no self loop)
    dinv = (1.0 / np.sqrt(deg + 1.0)).astype(np.float32)

    order = np.argsort(-deg, kind="stable")       # relabel: rank r -> orig order[r]
    rank_of = np.empty(N, np.int64)
    rank_of[order] = np.arange(N)
    core_of = rank_of % NC
    slot_of = rank_of // NC                       # 0..6249, degree-desc within core
    row_of = core_of * NPC + slot_of              # DRAM row in hhat_full

    per_core = []
    src_row = row_of[src]
    dst_core = core_of[dst]
    dst_slot = slot_of[dst]
    for c in range(NC):
        em = dst_core == c
        e_slot = dst_slot[em]
        e_srow = src_row[em]
        o = np.argsort(e_slot, kind="stable")
        e_slot = e_slot[o]
        e_srow = e_srow[o]
        counts = np.bincount(e_slot, minlength=NPC)
        starts = np.concatenate([[0], np.cumsum(counts)])
        tiles = []
        for t in range(NT):
            sl0 = t * 128
            nt = int(counts[sl0:sl0 + 128].max())
            rows_by_slot = []
            for p in range(128):
                s = sl0 + p
                rows_by_slot.append(e_srow[starts[s]:starts[s + 1]]
                                    if s < NPC else np.empty(0, np.int64))
            tiles.append((nt, rows_by_slot))
        per_core.append(tiles)
    return dinv, order, per_core


def _build_groups(nt_max):
    """Group tiles so total columns (incl +1 filler per group) <= MAXCOLS+1."""
    groups = []
    g_tiles, g_cols = [], 0
    for t in range(NT):
        nt = int(nt_max[t])
        if g_tiles and g_cols + nt > MAXCOLS:
            groups.append(g_tiles)
            g_tiles, g_cols = [], 0
        g_tiles.append(t)
        g_cols += nt
    if g_tiles:
        groups.append(g_tiles)
    return groups


def _build_idx(per_core_tiles, nt_max, groups_meta):
    """Flat int16 gather index stream for one core (shifted rows + fillers)."""
    parts = []
    for g in groups_meta:
        for t in g:
            ntu = int(nt_max[t])
            if ntu == 0:
                continue
            nt, rows_by_slot = per_core_tiles[t]
            grid = np.full((ntu, 128), ZIDX, np.int32)
            for p in range(128):
                rows = rows_by_slot[p]
                k = len(rows)
                if k:
                    grid[:k, p] = rows - SHIFT
            parts.append(grid.reshape(-1))
        parts.append(np.full(128, ZIDX, np.int32))   # filler column per group
    flat = np.concatenate(parts).astype(np.int16)
    assert flat[-1] >= 0
    wrap = np.zeros((128, len(flat) // 16), np.int16)
    a = flat.reshape(-1, 16)
    for gg in range(8):
        wrap[gg * 16:(gg + 1) * 16, :] = a.T
    return wrap, len(flat) // 128


def _build_program(tiles_meta, groups_meta, idx_cols16):
    """Build the bass program (same for all cores; per-core data via inputs)."""
    nc = bacc.Bacc("TRN2", target_bir_lowering=False, debug=False, num_devices=NC,
                   num_swdge_queues=4, dynamic_dma_scratch_size=32768)
    f32 = mybir.dt.float32
    xT_d = nc.dram_tensor("xT", [D_IN, NPC], f32, kind="ExternalInput")
    W1_d = nc.dram_tensor("W1", [D_IN, H], f32, kind="ExternalInput")
    W2_d = nc.dram_tensor("W2", [H, H], f32, kind="ExternalInput")
    W3_d = nc.dram_tensor("W3", [H, H], f32, kind="ExternalInput")
    bcol_d = nc.dram_tensor("bcol", [H, 3], f32, kind="ExternalInput")
    dinvc_d = nc.dram_tensor("dinvc", [128, NT], f32, kind="ExternalInput")
    idx_d = nc.dram_tensor("idx16", [128, idx_cols16], mybir.dt.int16, kind="ExternalInput")
    ident_d = nc.dram_tensor("ident", [128, 128], f32, kind="ExternalInput")
    fw1_d = nc.dram_tensor("fw1", [2 * H, H], f32, kind="ExternalInput")
    fb1_d = nc.dram_tensor("fb1", [H, 1], f32, kind="ExternalInput")
    fw2_d = nc.dram_tensor("fw2", [H, 1], f32, kind="ExternalInput")
    fb2_d = nc.dram_tensor("fb2", [1, 1], f32, kind="ExternalInput")
    out_d = nc.dram_tensor("out", [1, 1], f32, kind="ExternalOutput")

    with tile.TileContext(nc) as tc:
        with (
            tc.tile_pool(name="const", bufs=1) as cst,
            tc.tile_pool(name="hhat", bufs=1) as hhp,
            tc.tile_pool(name="yt", bufs=1) as ytp,
            tc.tile_pool(name="gb", bufs=3) as gbp,
            tc.tile_pool(name="acc", bufs=3) as accp,
            tc.tile_pool(name="ps", bufs=2, space="PSUM") as psp,
            tc.tile_pool(name="hdps", bufs=1, space="PSUM") as hdp,
            tc.tile_pool(name="zps", bufs=2, space="PSUM") as zpsp,
            tc.tile_pool(name="zsb", bufs=2) as zsbp,
            tc.tile_pool(name="dram", bufs=1, space="DRAM") as dram,
        ):
            # constants
            W1 = cst.tile([D_IN, H], f32)
            nc.sync.dma_start(out=W1[:], in_=W1_d[:])
            W2 = cst.tile([H, H], f32)
            nc.sync.dma_start(out=W2[:], in_=W2_d[:])
            W3 = cst.tile([H, H], f32)
            nc.sync.dma_start(out=W3[:], in_=W3_d[:])
            Wt = [W1, W2, W3]
            bcol = cst.tile([H, 3], f32)
            nc.sync.dma_start(out=bcol[:], in_=bcol_d[:])
            dinvc = cst.tile([128, NT], f32)
            nc.sync.dma_start(out=dinvc[:], in_=dinvc_d[:])
            idx16 = cst.tile([128, idx_cols16], mybir.dt.int16)
            nc.sync.dma_start(out=idx16[:], in_=idx_d[:])
            ident = cst.tile([128, 128], f32)
            nc.sync.dma_start(out=ident[:], in_=ident_d[:])
            fw1 = cst.tile([2 * H, H], f32)
            nc.sync.dma_start(out=fw1[:], in_=fw1_d[:])
            fb1 = cst.tile([H, 1], f32)
            nc.sync.dma_start(out=fb1[:], in_=fb1_d[:])
            fw2 = cst.tile([H, 1], f32)
            nc.sync.dma_start(out=fw2[:], in_=fw2_d[:])
            fb2 = cst.tile([1, 1], f32)
            nc.sync.dma_start(out=fb2[:], in_=fb2_d[:])

            xT = cst.tile([D_IN, NPC], f32)
            hh = [hhp.tile([128, NT * H], f32, name=f"hh{i}") for i in range(2)]
            yT = ytp.tile([H, NPC], f32)

            ag_in = [dram.tile([NPC, H], f32, name=f"agin{l}") for l in range(3)]
            ag_out = [dram.tile([NTOT, H], f32, addr_space="Shared", name=f"agout{l}")
                      for l in range(3)]

            def mm_chunk(l, rhs_sb, ch0, cw, hob):
                """One 512-col chunk of hhat_l = dinv * (W_l^T @ rhs); write to
                hh[hob] node-major and flush the chunk to ag_in[l]."""
                zps = zpsp.tile([H, 512], f32, tag="zps")
                nc.tensor.matmul(out=zps[:, :cw], lhsT=Wt[l][:],
                                 rhs=rhs_sb[:, ch0:ch0 + cw],
                                 start=True, stop=True)
                zsb = zsbp.tile([H, 512], f32, tag="zsb")
                nc.vector.tensor_copy(out=zsb[:, :cw], in_=zps[:, :cw])
                for q in range(0, cw, 128):
                    t = (ch0 + q) // 128
                    tp = psp.tile([128, H], f32, tag="tp")
                    nc.tensor.transpose(out=tp[:], in_=zsb[:, q:q + 128],
                                        identity=ident[:H, :H])
                    nc.vector.tensor_scalar(
                        out=hh[hob][:, t * H:(t + 1) * H], in0=tp[:],
                        scalar1=dinvc[:, t:t + 1], scalar2=None,
                        op0=mybir.AluOpType.mult)
                t0 = ch0 // 128
                ntc = cw // 128
                nc.sync.dma_start(
                    out=ag_in[l][:].rearrange("(t p) d -> p t d", p=128)
                        [:, t0:t0 + ntc, :],
                    in_=hh[hob][:, t0 * H:(t0 + ntc) * H]
                        .rearrange("p (t d) -> p t d", d=H))

            def launch_ag(l):
                nc.gpsimd.collective_compute(
                    "AllGather", mybir.AluOpType.bypass,
                    replica_groups=[list(range(NC))],
                    ins=[ag_in[l].opt()], outs=[ag_out[l].opt()])

            # ---- prologue: layer 0 mm over xT chunks (overlap load+matmul) ----
            for ch0 in range(0, NPC, 512):
                cw = min(512, NPC - ch0)
                nc.sync.dma_start(out=xT[:, ch0:ch0 + cw],
                                  in_=xT_d[:, ch0:ch0 + cw])
                mm_chunk(0, xT, ch0, cw, 0)
            launch_ag(0)

            sum_acc = accp.tile([H, 1], f32, tag="pool", name="sum_acc")
            max_acc = accp.tile([H, 1], f32, tag="pool", name="max_acc")

            def pool_chunk(ch0, cw, first):
                rs = accp.tile([H, 1], f32, tag="pool", name=f"rs{ch0}")
                rm = accp.tile([H, 1], f32, tag="pool", name=f"rm{ch0}")
                nc.vector.reduce_sum(out=rs[:], in_=yT[:, ch0:ch0 + cw],
                                     axis=mybir.AxisListType.X)
                nc.vector.reduce_max(out=rm[:], in_=yT[:, ch0:ch0 + cw],
                                     axis=mybir.AxisListType.X)
                if first:
                    nc.vector.tensor_copy(out=sum_acc[:], in_=rs[:])
                    nc.vector.tensor_copy(out=max_acc[:], in_=rm[:])
                else:
                    nc.vector.tensor_add(out=sum_acc[:], in0=sum_acc[:], in1=rs[:])
                    nc.vector.tensor_max(out=max_acc[:], in0=max_acc[:], in1=rm[:])

            def aggregate(l):
                """dst-tile aggregation from ag_out[l] into yT; next layer's mm
                chunks (or final pooling chunks) interleave after each 512 cols."""
                hob = l % 2
                src_view = ag_out[l][SHIFT:SHIFT + 128, :]
                col_off = 0
                gi = 0
                next_ch0 = 0
                for g_tiles in groups_meta:
                    cols = sum(tiles_meta[t] for t in g_tiles) + 1  # +filler
                    nidx = cols * 128
                    gb = gbp.tile([128, cols * H], f32, tag="gb",
                                  name=f"gb{l}_{g_tiles[0]}")
                    nc.gpsimd.dma_gather(
                        out_ap=gb[:].rearrange("p (n d) -> p n d", d=H),
                        in_ap=src_view,
                        idxs_ap=idx16[:, col_off * 8:(col_off + cols) * 8],
                        num_idxs=nidx, num_idxs_reg=nidx,
                        elem_size=H, single_packet=False,
                        queue_num=gi % 4)
                    gi += 1
                    seg = 0
                    for t in g_tiles:
                        nt = tiles_meta[t]
                        if nt == 0:
                            continue
                        gseg = gb[:, seg * H:(seg + nt) * H]
                        acc = accp.tile([128, H], f32, tag="acc", name=f"acc{l}_{t}")
                        nc.vector.tensor_reduce(
                            out=acc[:],
                            in_=gseg.rearrange("p (cs d) -> p d cs", d=H),
                            axis=mybir.AxisListType.X, op=mybir.AluOpType.add)
                        nc.vector.tensor_add(out=acc[:], in0=acc[:],
                                             in1=hh[hob][:, t * H:(t + 1) * H])
                        nc.vector.tensor_scalar(
                            out=acc[:], in0=acc[:], scalar1=dinvc[:, t:t + 1],
                            scalar2=None, op0=mybir.AluOpType.mult)
                        yps = psp.tile([H, 128], f32, tag="yps", name=f"yps{l}_{t}")
                        nc.tensor.transpose(out=yps[:], in_=acc[:], identity=ident[:])
                        nc.scalar.activation(
                            out=yT[:, t * 128:(t + 1) * 128], in_=yps[:],
                            func=mybir.ActivationFunctionType.Relu,
                            bias=bcol[:, l:l + 1])
                        seg += nt
                        # interleave next layer's mm (or pooling) on done tiles
                        if l < 2:
                            while next_ch0 + 512 <= (t + 1) * 128:
                                mm_chunk(l + 1, yT, next_ch0, 512, 1 - hob)
                                next_ch0 += 512
                        else:
                            while (next_ch0 + 512 <= (t + 1) * 128
                                   and next_ch0 + 512 <= 6144):
                                pool_chunk(next_ch0, 512, next_ch0 == 0)
                                next_ch0 += 512
                    col_off += cols
                if l < 2:
                    while next_ch0 < NPC:
                        cw = min(512, NPC - next_ch0)
                        mm_chunk(l + 1, yT, next_ch0, cw, 1 - hob)
                        next_ch0 += cw
                    launch_ag(l + 1)
                else:
                    # zero pad columns (slots 6250..6271), pool the tail chunk
                    nc.vector.memset(yT[:, NPC - 22:], 0.0)
                    pool_chunk(6144, NPC - 6144, False)

            aggregate(0)
            aggregate(1)
            aggregate(2)

            pool2 = accp.tile([H, 2], f32, tag="pool", name="pool2")
            nc.vector.tensor_copy(out=pool2[:, 0:1], in_=sum_acc[:])
            nc.vector.tensor_copy(out=pool2[:, 1:2], in_=max_acc[:])
            agp_in = dram.tile([H, 2], f32, name="agpin")
            agp_out = dram.tile([NC * H, 2], f32, addr_space="Shared", name="agpout")
            nc.sync.dma_start(out=agp_in[:], in_=pool2[:])
            nc.gpsimd.collective_compute(
                "AllGather", mybir.AluOpType.bypass,
                replica_groups=[list(range(NC))],
                ins=[agp_in.opt()], outs=[agp_out.opt()])
            allp = accp.tile([H, 2 * NC], f32, tag="allp", name="allp")
            nc.sync.dma_start(
                out=allp[:].rearrange("p (r d) -> p r d", d=2),
                in_=agp_out[:].rearrange("(r p) d -> p r d", p=H))
            gsum = accp.tile([H, 1], f32, tag="pool", name="gsum")
            gmax = accp.tile([H, 1], f32, tag="pool", name="gmax")
            nc.vector.reduce_sum(
                out=gsum[:], in_=allp[:].rearrange("p (r d) -> p d r", d=2)[:, 0:1, :],
                axis=mybir.AxisListType.X)
            nc.vector.reduce_max(
                out=gmax[:], in_=allp[:].rearrange("p (r d) -> p d r", d=2)[:, 1:2, :],
                axis=mybir.AxisListType.X)
            nc.vector.tensor_scalar(out=gsum[:], in0=gsum[:], scalar1=1.0 / N,
                                    scalar2=None, op0=mybir.AluOpType.mult)
            pooled = accp.tile([2 * H, 1], f32, tag="pooled", name="pooled")
            nc.sync.dma_start(out=pooled[:H, :], in_=gsum[:])
            nc.sync.dma_start(out=pooled[H:, :], in_=gmax[:])
            h1ps = hdp.tile([H, 1], f32, tag="hd", name="h1ps")
            nc.tensor.matmul(out=h1ps[:], lhsT=fw1[:], rhs=pooled[:],
                             start=True, stop=True)
            r1 = accp.tile([H, 1], f32, tag="pool", name="r1")
            nc.scalar.activation(out=r1[:], in_=h1ps[:],
                                 func=mybir.ActivationFunctionType.Relu,
                                 bias=fb1[:, 0:1])
            h2ps = hdp.tile([1, 1], f32, tag="hd2", name="h2ps")
            nc.tensor.matmul(out=h2ps[:], lhsT=fw2[:], rhs=r1[:],
                             start=True, stop=True)
            ores = accp.tile([1, 1], f32, tag="ores", name="ores")
            nc.vector.tensor_add(out=ores[:], in0=h2ps[:],
                                 in1=fb2[:, 0:1])
            nc.sync.dma_start(out=out_d[:], in_=ores[:])

    nc.compile()
    return nc


def kernel(x, edge_index, W1, b1, W2, b2, W3, b3, fw1, fb1, fw2, fb2):
    dinv, order, per_core = _preprocess(edge_index)

    # Unified per-tile column counts across cores (one SPMD program).
    nts = np.zeros((NC, NT), np.int64)
    for c in range(NC):
        for t in range(NT):
            nts[c, t] = per_core[c][t][0]
    nt_max = nts.max(axis=0)
    tiles_meta = [int(nt_max[t]) for t in range(NT)]
    groups_meta = _build_groups(nt_max)
    total_cols = int(nt_max.sum()) + len(groups_meta)   # + fillers
    idx_cols16 = total_cols * 8

    b3col = np.stack([np.asarray(b1, np.float32), np.asarray(b2, np.float32),
                      np.asarray(b3, np.float32)], axis=1)  # [64, 3]
    ident = np.eye(128, dtype=np.float32)

    in_maps = []
    x = np.asarray(x, np.float32)
    for c in range(NC):
        ranks = np.arange(c, N, NC)
        orig = order[ranks]
        xT = np.zeros((D_IN, NPC), np.float32)
        xT[:, :len(orig)] = x[orig].T
        dv = np.zeros(NPC, np.float32)
        dv[:len(orig)] = dinv[orig]
        dinv_col = dv.reshape(NT, 128).T.copy()      # [128, 49]
        wrap, ncols = _build_idx(per_core[c], nt_max, groups_meta)
        assert ncols == total_cols, (ncols, total_cols)
        in_maps.append({
            "xT": xT, "W1": np.asarray(W1, np.float32),
            "W2": np.asarray(W2, np.float32), "W3": np.asarray(W3, np.float32),
            "bcol": b3col, "dinvc": dinv_col, "idx16": wrap,
            "ident": ident, "fw1": np.asarray(fw1, np.float32),
            "fb1": np.asarray(fb1, np.float32).reshape(H, 1),
            "fw2": np.asarray(fw2, np.float32).reshape(H, 1),
            "fb2": np.asarray(fb2, np.float32).reshape(1, 1),
        })

    nc = _build_program(tiles_meta, groups_meta, idx_cols16)

    trace = os.environ.get("BASS_GCN_TRACE", "0") == "1"
    if trace:
        trace = _install_trace_hook()
    res = bass_utils.run_bass_kernel_spmd(
        nc, in_maps, core_ids=list(range(NC)), trace=trace)
    _EXEC_NS[0] = res.exec_time_ns
    out = res.results[0]["out"]
    return np.asarray(out, np.float32).reshape(1, 1)

